# revision 1
# baseline (speedup 1.0000x reference)
"""Trainium2 Bass kernel for nn_Density: radial-flow mixture log-density.

Computes log q(z|c) for a 6-layer batched radial normalizing flow with a
standard-normal base, for C=16 classes over N=200000 samples, data-parallel
over 8 NeuronCores.

Math: the radial update z' = z + beta*h*(z - z0) with h = 1/(alpha + r),
r = ||z - z0||, is, per (sample, class), a scalar rescaling of z_sub = z - z0:
    z_sub_{l+1} = g_l * z_sub_l + Delta_l,   g_l = 1 + beta_l*h_l,
    Delta_l = z0_l - z0_{l+1}  (Delta_5 = z0_5, so z_sub_6 = z_final).
So r^2 and every needed dot product obey cheap scalar recurrences:
    r2'   = g*(g*r2 + 2*e_l) + ||Delta_l||^2
    e_m'  = g*e_m + Delta_l . Delta_m        (e_m = z_sub . Delta_m)
log|det J| terms accumulate as running products, logged once at the end:
    slj = 15*ln(prod g_l) + ln(prod (1 + alpha_l*beta_l*h_l^2)).

Layout: partitions hold (class, sample-block) pairs: p = c*8 + s, so every
per-class constant is a per-partition scalar ([128,1] AP) usable by
tensor_scalar two-op fusions and ACT scale/bias. The free axis holds FN
samples. Block-sparse stationary matmuls lhsT[(d,s8),(c,s)] = W[d,c]*δ(s8,s)
seed r2_0 = ||z||^2 - 2 z.z0_0 (+c1 folded into the PSUM copy) and
e_m = z.Delta_m (- z0_0.Delta_m folded into the copy) directly in PSUM.
The host untangles the (c,s)-partition output ordering for free.
"""

from contextlib import ExitStack

import numpy as np

import concourse.bacc as bacc
import concourse.bass as bass
import concourse.mybir as mybir
import concourse.tile as tile
from concourse.bass_utils import run_bass_kernel_spmd

F32 = mybir.dt.float32
F16 = mybir.dt.float16
A = mybir.AluOpType
ACTF = mybir.ActivationFunctionType

N, C, DIM, L = 200000, 16, 16, 6
NCORES = 8
SB = 8                      # sample blocks per class on partitions
FN = 448                    # samples per partition slot (free axis)
NG = SB * FN                # 3584 samples per group
GROUPS = 7
NC_SAMP = N // NCORES       # 25000
NC_PAD = NG * GROUPS        # 25088

# const blob column indices ([128, NCONST] f32, value = f(class(p)))
IDX_A = 0          # alpha_l         -> 0..5
IDX_B = 6          # beta_l          -> 6..11
IDX_AB = 12        # alpha_l*beta_l  -> 12..17
IDX_K = 18         # ||Delta_l||^2   -> 18..23
IDX_C1 = 24        # ||z0_0||^2
IDX_S = 25         # -(z0_0 . Delta_m)  -> 25..30   (sign pre-folded)
IDX_DD = 31        # Delta_l . Delta_m, (0,1)..(0,5),(1,2)..(4,5) -> 31..45
IDX_KC = 46        # -0.5*||Delta_5||^2 - 8*ln(2pi)  (tail fold)
NCONST = 47

_PAIR_IDX = {}
_p = 0
for _l in range(L):
    for _m in range(_l + 1, L):
        _PAIR_IDX[(_l, _m)] = _p
        _p += 1

LOG2PI = float(np.log(2.0 * np.pi))


def _host_consts(z0, log_alpha, beta):
    """Build stationary blocks [8, 128, 128] and const blob [128, NCONST]."""
    z0 = z0.astype(np.float32)
    alpha = np.exp(log_alpha.astype(np.float32)).astype(np.float32)
    beta = beta.astype(np.float32)
    delta = np.concatenate([z0[:-1] - z0[1:], z0[-1:]], axis=0).astype(np.float32)

    # wcols[m]: [DIM, C];  m=0 -> -2*z0_0 (r2 seed), m=1..6 -> Delta_{m-1}
    wcols = np.zeros((7, DIM, C), np.float32)
    wcols[0] = -2.0 * z0[0].T
    for m in range(L):
        wcols[m + 1] = delta[m].T

    # stationary blocks: blk[j][(d*8+s8), (c*8+s)] = wcols[j][d,c]*δ(s8,s);
    # blk[7] = ones-block (Q accumulation into the r2 seed).
    blocks = np.zeros((8, 128, 128), np.float32)
    eye8 = np.eye(SB, dtype=np.float32)
    for j in range(7):
        blocks[j] = np.einsum("dc,st->dsct", wcols[j], eye8).reshape(128, 128)
    blocks[7] = np.einsum("dc,st->dsct",
                          np.ones((DIM, C), np.float32), eye8).reshape(128, 128)

    cst = np.zeros((NCONST, C), np.float32)
    for l in range(L):
        cst[IDX_A + l] = alpha[l]
        cst[IDX_B + l] = beta[l]
        cst[IDX_AB + l] = alpha[l] * beta[l]
        cst[IDX_K + l] = np.sum(delta[l] ** 2, axis=-1)
    cst[IDX_C1] = np.sum(z0[0] ** 2, axis=-1)
    for m in range(L):
        cst[IDX_S + m] = -np.einsum("cd,cd->c", z0[0], delta[m])
    for (l, m), p in _PAIR_IDX.items():
        cst[IDX_DD + p] = np.einsum("cd,cd->c", delta[l], delta[m])
    cst[IDX_KC] = -0.5 * cst[IDX_K + L - 1] - np.float32(8.0 * LOG2PI)

    # blob[p, i] = cst[i, class(p)],  class(p) = p // 8
    blob = cst.T[np.repeat(np.arange(C), SB)].copy()  # [128, NCONST]
    return blocks, blob


def _build_program(reps=1):
    nc = bacc.Bacc("TRN2", target_bir_lowering=False, debug=False,
                   num_devices=NCORES)
    zd_d = nc.dram_tensor("zd", [GROUPS, 128, FN], F32, kind="ExternalInput")
    wb_d = nc.dram_tensor("wb", [8, 128, 128], F32, kind="ExternalInput")
    cst_d = nc.dram_tensor("cst", [128, NCONST], F32, kind="ExternalInput")
    out_d = nc.dram_tensor("out", [GROUPS, 128, FN], F32, kind="ExternalOutput")

    with tile.TileContext(nc) as tc, ExitStack() as ctx:
        const_pool = ctx.enter_context(tc.tile_pool(name="const", bufs=1))
        wbt = const_pool.tile([128, 8 * 128], F32)
        for j in range(8):
            nc.sync.dma_start(wbt[:, j * 128:(j + 1) * 128], wb_d[j])
        cst = const_pool.tile([128, NCONST], F32)
        nc.sync.dma_start(cst[:], cst_d[:])

        def wb(j):
            return wbt[:, j * 128:(j + 1) * 128]

        def ca(i):
            return cst[:, i:i + 1]            # [128,1] per-partition const


        io_pool = ctx.enter_context(tc.tile_pool(name="io", bufs=3))
        e_pool = ctx.enter_context(tc.tile_pool(name="e", bufs=1))
        st_pool = ctx.enter_context(tc.tile_pool(name="st", bufs=2))
        tmp_pool = ctx.enter_context(tc.tile_pool(name="tmp", bufs=2))
        fin_pool = ctx.enter_context(tc.tile_pool(name="fin", bufs=1))
        ps_pool = ctx.enter_context(tc.tile_pool(name="ps", bufs=1, space="PSUM"))
        ps2_pool = ctx.enter_context(tc.tile_pool(name="ps2", bufs=2, space="PSUM"))

        finals = []
        for _rep in range(reps):
         for g in range(GROUPS):
            zd = io_pool.tile([128, FN], F32, tag="zd")
            nc.sync.dma_start(zd[:], zd_d[g])
            zsq = tmp_pool.tile([128, FN], F32, tag=f"zsq{g % 3}")
            nc.scalar.activation(zsq[:], zd[:], ACTF.Square)

            # r2 seed: psum = (-2 z0_0-block) @ zd + ones-block @ zsq
            r2p = ps2_pool.tile([128, FN], F32, tag="r2p")
            nc.tensor.matmul(r2p[:], wb(0), zd[:], start=True, stop=False)
            nc.tensor.matmul(r2p[:], wb(7), zsq[:], start=False, stop=True)
            # e_m seeds
            eps = []
            for m in range(L):
                ep = ps_pool.tile([128, FN], F32, tag=f"ep{m}")
                nc.tensor.matmul(ep[:], wb(m + 1), zd[:], start=True, stop=True)
                eps.append(ep)

            # r2 stays "pre-bias": the +c1/+k_l constant rides the next
            # Sqrt's bias and the t1 STT; layer 0 reads the PSUM seed directly
            r2 = r2p
            e_all = e_pool.tile([128, L * FN], F16, tag=f"e{g % 3}")
            for m in range(L):
                nc.scalar.activation(e_all[:, m * FN:(m + 1) * FN],
                                     eps[m][:], ACTF.Identity,
                                     bias=ca(IDX_S + m))

            def e(m):
                return e_all[:, m * FN:(m + 1) * FN]

            gp = fin_pool.tile([128, FN], F32, tag=f"gp{g}")
            pp = fin_pool.tile([128, FN], F32, tag=f"pp{g}")

            for l in range(L):
                bias_idx = IDX_C1 if l == 0 else IDX_K + l - 1
                r = tmp_pool.tile([128, FN], F32, tag=f"r{g % 3}")
                nc.scalar.activation(r[:], r2[:], ACTF.Sqrt, bias=ca(bias_idx))
                hd = tmp_pool.tile([128, FN], F32, tag=f"hd{g % 3}")
                if g % 2 == 0:
                    nc.scalar.activation(hd[:], r[:], ACTF.Identity,
                                         bias=ca(IDX_A + l))
                else:
                    nc.vector.tensor_scalar(hd[:], r[:], ca(IDX_A + l),
                                            None, A.add)
                h = tmp_pool.tile([128, FN], F32, tag=f"h{g % 3}")
                nc.vector.reciprocal_approx_fast(h[:], hd[:])
                g_ = tmp_pool.tile([128, FN], F32, tag=f"g_{g % 3}")
                nc.scalar.activation(g_[:], h[:], ACTF.Identity,
                                     bias=1.0, scale=ca(IDX_B + l))
                if l < L - 1:
                    g16 = tmp_pool.tile([128, FN], F16, tag=f"g16{g % 3}")
                    nc.scalar.activation(g16[:], h[:], ACTF.Identity,
                                         bias=1.0, scale=ca(IDX_B + l))

                # log-det products (off critical path, Pool does only TT/copy
                # -- TensorScalarPtr is not a legal Pool opcode).
                # 1 + ab*h^2 == h*(hd + ab*h) == h*(alpha*g + r).
                if l == 0:
                    nc.gpsimd.tensor_copy(gp[:], g_[:])
                else:
                    nc.gpsimd.tensor_tensor(gp[:], gp[:], g_[:], A.mult)
                va = tmp_pool.tile([128, FN], F32, tag=f"va{g % 3}")
                nc.vector.tensor_scalar(va[:], g_[:], ca(IDX_A + l), None,
                                        A.mult)
                v = tmp_pool.tile([128, FN], F32, tag=f"v{g % 3}")
                nc.gpsimd.tensor_tensor(v[:], va[:], r[:], A.add)
                u1 = tmp_pool.tile([128, FN], F32, tag=f"u1{g % 3}")
                nc.gpsimd.tensor_tensor(u1[:], h[:], v[:], A.mult)
                if l == 0:
                    nc.gpsimd.tensor_copy(pp[:], u1[:])
                else:
                    nc.gpsimd.tensor_tensor(pp[:], pp[:], u1[:], A.mult)

                # r2' = g*((r2 + bias) * g ... ) with the +k fold:
                # t1 = (r2 + bias)*g;  t4 = 2*e_l + t1;  r2_next = g*t4 (pre-k)
                t1 = tmp_pool.tile([128, FN], F32, tag=f"t1{g % 3}")
                nc.vector.scalar_tensor_tensor(t1[:], r2[:], ca(bias_idx),
                                               g_[:], A.add, A.mult)
                t4 = tmp_pool.tile([128, FN], F32, tag=f"t4{g % 3}")
                nc.vector.scalar_tensor_tensor(t4[:], e(l), 2.0, t1[:],
                                               A.mult, A.add)
                if l == L - 1:
                    r2n = fin_pool.tile([128, FN], F32, tag=f"r2f{g}")
                else:
                    r2n = st_pool.tile([128, FN], F32, tag=f"r2{g % 3}")
                last_body_inst = nc.vector.tensor_tensor(
                    r2n[:], g_[:], t4[:], A.mult)
                r2 = r2n

                # e_m' = g*e_m + DD[l][m]: one bulk fp16 mult over the
                # contiguous m>l slab (2x mode -- innermost dims stay
                # contiguous), then per-m 4x TS adds
                if l < L - 1:
                    nm = L - 1 - l
                    esl = (e_all[:, (l + 1) * FN: L * FN]
                           .rearrange("p (m f) -> p m f", m=nm))
                    gb = (g16.rearrange("p (o f) -> p o f", o=1)
                          .to_broadcast((128, nm, FN)))
                    nc.vector.tensor_tensor(esl, esl, gb, A.mult)
                    for m in range(l + 1, L):
                        nc.vector.tensor_scalar(
                            e(m), e(m), ca(IDX_DD + _PAIR_IDX[(l, m)]),
                            None, A.add)

            finals.append((gp, pp, r2))

        # Tail: batched Ln's + final combine.  Explicit deps pin every Ln
        # after the last group's body so the Sqrt<->Ln ACT table switch
        # happens exactly once.  (reps>1 is a timing-only mode; only the
        # last rep's results are finalized.)
        finals = finals[-GROUPS:]
        from concourse.tile_rust import add_dep_helper
        for g, (gp, pp, r2) in enumerate(finals):
            lg = tmp_pool.tile([128, FN], F32, tag="lg")
            i1 = nc.scalar.activation(lg[:], gp[:], ACTF.Ln)
            lp = tmp_pool.tile([128, FN], F32, tag="lp")
            i2 = nc.scalar.activation(lp[:], pp[:], ACTF.Ln)
            add_dep_helper(i1.ins, last_body_inst.ins,
                           sync=True, reason="batch Ln after all Sqrt")
            add_dep_helper(i2.ins, last_body_inst.ins,
                           sync=True, reason="batch Ln after all Sqrt")
            t5 = tmp_pool.tile([128, FN], F32, tag="t5")
            nc.vector.scalar_tensor_tensor(t5[:], lg[:], 15.0, lp[:],
                                           A.mult, A.add)
            t6 = tmp_pool.tile([128, FN], F32, tag="t6")
            nc.vector.tensor_scalar(t6[:], r2[:], -0.5, ca(IDX_KC),
                                    A.mult, A.add)
            ot = io_pool.tile([128, FN], F32, tag="ot")
            nc.vector.tensor_tensor(ot[:], t5[:], t6[:], A.add)
            nc.sync.dma_start(out_d[g], ot[:])

    nc.compile()
    return nc


_NC_CACHE = None


def _get_nc():
    global _NC_CACHE
    if _NC_CACHE is None:
        _NC_CACHE = _build_program()
    return _NC_CACHE


def _prepare_in_maps(z, z0, log_alpha, beta):
    blocks, blob = _host_consts(z0, log_alpha, beta)
    z = np.ascontiguousarray(z.astype(np.float32))
    in_maps = []
    for c in range(NCORES):
        shard = z[c * NC_SAMP:(c + 1) * NC_SAMP]
        pad = np.zeros((NC_PAD, DIM), np.float32)
        pad[:NC_SAMP] = shard
        # zd[g, d*8+s8, f] = z[g*NG + s8*FN + f, d]
        cube = pad.reshape(GROUPS, SB, FN, DIM)
        zd = np.ascontiguousarray(
            cube.transpose(0, 3, 1, 2).reshape(GROUPS, 128, FN))
        in_maps.append({"zd": zd, "wb": blocks, "cst": blob})
    return in_maps


def _gather_out(raw):
    """raw [GROUPS, 128=(c*8+s), FN] -> [NC_PAD, C] in sample order."""
    # raw[g, c*8+s, f] = logq(n = g*NG + s*FN + f, c)
    r = raw.reshape(GROUPS, C, SB, FN)
    return r.transpose(0, 2, 3, 1).reshape(NC_PAD, C)


def _numpy_fallback(z, z0, log_alpha, beta, mean, cov):
    # General mean/cov path (never hit for this problem's fixed buffers).
    z = z.astype(np.float32)
    zc = np.broadcast_to(z[None], (C,) + z.shape).astype(np.float32)
    slj = np.zeros((C, z.shape[0]), np.float32)
    alpha = np.exp(log_alpha.astype(np.float32))
    zk = zc.copy()
    for l in range(L):
        z_sub = zk - z0[l][:, None, :]
        r = np.linalg.norm(z_sub, axis=-1, keepdims=True)
        h = 1.0 / (alpha[l][:, None, None] + r)
        b = beta[l][:, None, None]
        zk = zk + b * h * z_sub
        bh = b * h
        ld = (DIM - 1) * np.log1p(bh) + np.log1p(bh - b * r * h * h)
        slj += ld[..., 0]
    Lc = np.linalg.cholesky(cov)
    diff = zk - mean[:, None, :]
    sol = np.einsum("cij,cnj->cni", np.linalg.inv(Lc), diff)
    half_logdet = np.sum(np.log(np.diagonal(Lc, axis1=-2, axis2=-1)), axis=-1)
    lpz = -0.5 * (DIM * LOG2PI + np.sum(sol * sol, axis=-1)) \
        - half_logdet[:, None]
    out = (lpz + slj).T.astype(np.float32)
    return np.where(np.isnan(out), -np.inf, out)


def kernel(z, z0, log_alpha, beta, mean, cov):
    z = np.asarray(z)
    z0 = np.asarray(z0)
    log_alpha = np.asarray(log_alpha)
    beta = np.asarray(beta)
    mean = np.asarray(mean)
    cov = np.asarray(cov)
    if (not np.all(mean == 0.0)
            or not np.array_equal(cov, np.broadcast_to(np.eye(DIM, dtype=cov.dtype),
                                                       cov.shape))):
        return _numpy_fallback(z, z0, log_alpha, beta, mean, cov)

    try:
        nc = _get_nc()
        in_maps = _prepare_in_maps(z, z0, log_alpha, beta)
        res = run_bass_kernel_spmd(nc, in_maps, list(range(NCORES)))
        outs = []
        for c in range(NCORES):
            o = _gather_out(res.results[c]["out"])[:NC_SAMP]
            outs.append(o)
        out = np.concatenate(outs, axis=0).astype(np.float32)
    except Exception:
        # Device path unavailable (missing cores, wedged runtime, ...):
        # return the exact-but-slow host result instead of crashing.
        return _numpy_fallback(z, z0, log_alpha, beta, mean, cov)
    return np.where(np.isnan(out), np.float32(-np.inf), out)



# revision 3
# speedup vs baseline: 1.1668x; 1.1668x over previous
"""Trainium2 Bass kernel for nn_Density: radial-flow mixture log-density.

Computes log q(z|c) for a 6-layer batched radial normalizing flow with a
standard-normal base, for C=16 classes over N=200000 samples, data-parallel
over 8 NeuronCores.

Math: the radial update z' = z + beta*h*(z - z0) with h = 1/(alpha + r),
r = ||z - z0||, is, per (sample, class), a scalar rescaling of z_sub = z - z0:
    z_sub_{l+1} = g_l * z_sub_l + Delta_l,   g_l = 1 + beta_l*h_l,
so r^2 obeys a scalar recurrence driven by e_l = 2*z_sub.Delta_l:
    r2' = g*(g*r2 + e_l) + k_l,   k_l = ||Delta_l||^2.
The e_l recurrences are *approximated*: with B_l = prod_{i<l} g_i,
    e_l(l) ~= B_l*E_l + 2*Delta_{l-1}.Delta_l,
    E_l = 2*z_sub_0.Delta_l + 2*sum_{j<=l-2} Delta_j.Delta_l,
i.e. non-consecutive Delta.Delta cross terms are folded into the matmul
seed E_l (exact weight would be prod_{j<i<l} g_i ~= B_l); the consecutive
term keeps its exact weight 1.  Validated: max rel err ~5e-3 in fp16
(tolerance 2e-2).

Per layer only sqrt / divide / pow(-2) / mult / add are needed:
    r = sqrt(r2), hd = r+alpha, g = (hd+beta)/hd,
    u = 1 + (alpha*beta)*hd^-2,   (log-det: slj = 15*ln(prod g) + ln(prod u))
    r2' = g*(g*r2 + B*E + 2dd) + k.
No reciprocal, no Ln on device: gp = prod g, pp = prod u and the final r2
are DMA'd out in fp16 and the host computes
    out = -0.5*(r2+k5) + 15*ln(gp) + ln(pp) - 8*ln(2pi).
ACT uses only Sqrt/Square/Identity -> single act table, zero table switches.

Layout: partitions hold (class, sample-block) pairs p = c*8 + s, so every
per-class constant is a per-partition scalar ([128,1] AP).  The free axis
holds W=784 samples per supergroup (4 supergroups = 25088 >= 25000 samples
per core).  Seeds come from fp16 block-sparse stationary matmuls
lhsT[(d,s8),(c,s)] = w[d,c]*delta(s8,s) over zd/zsq, written as 392-wide
bank-aligned chunk pairs in [128,1024] PSUM tiles and evacuated by single
two-run ACT copies that fold the per-class seed bias.

Engine split per layer: ACT does the Sqrt (k rides the bias); DVE does the
cheap fp16 4x tensor-scalar ops and 2x tensor-tensor ops (incl. divide);
Pool/GPSIMD takes three fused scalar-tensor-tensor ops: t1 = (r2+k)*g,
pp' = (w+1)*pp and gp' = (gp*1)*g.
"""

from contextlib import ExitStack

import numpy as np

import concourse.bacc as bacc
import concourse.bass as bass
import concourse.mybir as mybir
import concourse.tile as tile
from concourse.bass_utils import run_bass_kernel_spmd

F32 = mybir.dt.float32
F16 = mybir.dt.float16
A = mybir.AluOpType
ACTF = mybir.ActivationFunctionType

N, C, DIM, L = 200000, 16, 16, 6
NCORES = 8
SB = 8                      # sample blocks per class on partitions
W = 784                     # samples per partition slot per supergroup
HW_ = 392                   # matmul chunk width (bank-aligned pairs)
SGROUPS = 4
NC_SAMP = N // NCORES       # 25000
NC_PAD = SB * W * SGROUPS   # 25088

# const blob column indices ([128, NCONST] f32, value = f(class(p)))
IDX_AL = 0          # alpha_l            -> 0..5
IDX_APB = 6         # alpha_l + beta_l   -> 6..11
IDX_AB = 12         # alpha_l * beta_l   -> 12..17
IDX_K = 18          # k_l = ||Delta_l||^2 -> 18..23
IDX_DD1 = 24        # 2*Delta_{l-1}.Delta_l, l=1..5 -> 24..28
IDX_C1 = 29         # ||z0_0||^2
IDX_EB = 30         # E_m seed bias -> 30..35
NCONST = 36

LOG2PI = float(np.log(2.0 * np.pi))


def _host_consts(z0, log_alpha, beta):
    """Build fp16 stationary blocks [8, 128, 128] and const blob [128, NCONST]."""
    z0 = z0.astype(np.float64)
    alpha = np.exp(log_alpha.astype(np.float64))
    beta = beta.astype(np.float64)
    delta = np.concatenate([z0[:-1] - z0[1:], z0[-1:]], axis=0)

    # wcols[m]: [DIM, C]; m=0 -> -2*z0_0 (r2 seed), m=1..6 -> 2*Delta_{m-1},
    # m=7 -> ones (zsq accumulation)
    wcols = np.zeros((8, DIM, C))
    wcols[0] = -2.0 * z0[0].T
    for m in range(L):
        wcols[m + 1] = 2.0 * delta[m].T
    wcols[7] = 1.0

    blocks = np.zeros((8, 128, 128), np.float16)
    eye8 = np.eye(SB)
    for j in range(8):
        blocks[j] = np.einsum("dc,st->dsct", wcols[j], eye8).reshape(128, 128)

    k = np.sum(delta ** 2, axis=-1)                        # [L, C]
    dd = np.einsum("lcd,mcd->lmc", delta, delta)           # [L, L, C]
    cst = np.zeros((NCONST, C))
    for l in range(L):
        cst[IDX_AL + l] = alpha[l]
        cst[IDX_APB + l] = alpha[l] + beta[l]
        cst[IDX_AB + l] = alpha[l] * beta[l]
        cst[IDX_K + l] = k[l]
    for l in range(1, L):
        cst[IDX_DD1 + l - 1] = 2.0 * dd[l - 1, l]
    cst[IDX_C1] = np.sum(z0[0] ** 2, axis=-1)
    for m in range(L):
        eb = -2.0 * np.einsum("cd,cd->c", z0[0], delta[m])
        if m >= 2:
            eb = eb + 2.0 * np.sum(dd[:m - 1, m], axis=0)
        cst[IDX_EB + m] = eb

    # blob[p, i] = cst[i, class(p)],  class(p) = p // 8
    blob = cst.T[np.repeat(np.arange(C), SB)].astype(np.float32).copy()
    return blocks, blob


def _build_program(reps=1):
    nc = bacc.Bacc("TRN2", target_bir_lowering=False, debug=False,
                   num_devices=NCORES)
    zd_d = nc.dram_tensor("zd", [SGROUPS, 128, W], F16, kind="ExternalInput")
    wb_d = nc.dram_tensor("wb", [8, 128, 128], F16, kind="ExternalInput")
    cst_d = nc.dram_tensor("cst", [128, NCONST], F32, kind="ExternalInput")
    r2_d = nc.dram_tensor("r2o", [SGROUPS, 128, W], F16, kind="ExternalOutput")
    gp_d = nc.dram_tensor("gpo", [SGROUPS, 128, W], F16, kind="ExternalOutput")
    pp_d = nc.dram_tensor("ppo", [SGROUPS, 128, W], F16, kind="ExternalOutput")

    with tile.TileContext(nc) as tc, ExitStack() as ctx:
        const_pool = ctx.enter_context(tc.tile_pool(name="const", bufs=1))
        wbt = const_pool.tile([128, 8 * 128], F16)
        for j in range(8):
            nc.sync.dma_start(wbt[:, j * 128:(j + 1) * 128], wb_d[j])
        cst = const_pool.tile([128, NCONST], F32)
        nc.sync.dma_start(cst[:], cst_d[:])

        def wb(j):
            return wbt[:, j * 128:(j + 1) * 128]

        def ca(i):
            return cst[:, i:i + 1]            # [128,1] per-partition const

        io_pool = ctx.enter_context(tc.tile_pool(name="io", bufs=2))
        e_pool = ctx.enter_context(tc.tile_pool(name="e", bufs=2))
        st_pool = ctx.enter_context(tc.tile_pool(name="st", bufs=3))
        psr_pool = ctx.enter_context(tc.tile_pool(name="psr", bufs=1, space="PSUM"))
        pse_pool = ctx.enter_context(tc.tile_pool(name="pse", bufs=1, space="PSUM"))

        def two_run(t):
            """[128, 1024] psum tile -> [128, 2, 392] AP (the used chunks)."""
            return t.rearrange("p (r f) -> p r f", r=2)[:, :, 0:HW_]

        for _rep in range(reps):
         for sg in range(SGROUPS):
            zd = io_pool.tile([128, W], F16, tag="zd")
            nc.sync.dma_start(zd[:], zd_d[sg])
            zsq = io_pool.tile([128, W], F16, tag="zsq")
            nc.scalar.activation(zsq[:], zd[:], ACTF.Square)

            # ---- seeds ----------------------------------------------------
            # r2p: (-2 z0_0)-block @ zd + ones-block @ zsq, 392-chunk pairs
            r2p = psr_pool.tile([128, 1024], F32, tag="r2p")
            for h in range(2):
                sl = slice(512 * h, 512 * h + HW_)
                zsl = slice(HW_ * h, HW_ * (h + 1))
                nc.tensor.matmul(r2p[:, sl], wb(0), zd[:, zsl],
                                 start=True, stop=False)
                nc.tensor.matmul(r2p[:, sl], wb(7), zsq[:, zsl],
                                 start=False, stop=True)
            # E_m seeds
            eps = []
            for m in range(L):
                ep = pse_pool.tile([128, 1024], F32, tag=f"ep{m % 3}")
                for h in range(2):
                    sl = slice(512 * h, 512 * h + HW_)
                    zsl = slice(HW_ * h, HW_ * (h + 1))
                    nc.tensor.matmul(ep[:, sl], wb(m + 1), zd[:, zsl],
                                     start=True, stop=True)
                eps.append(ep)

            # evacuate E seeds to fp16 SBUF, folding the per-class bias
            e_all = e_pool.tile([128, L * W], F16, tag="e")

            def e(m):
                return e_all[:, m * W:(m + 1) * W]

            for m in range(L):
                nc.scalar.activation(
                    e(m).rearrange("p (r f) -> p r f", r=2), two_run(eps[m]),
                    ACTF.Identity, bias=ca(IDX_EB + m))

            # layer-0 r2 (true, incl. +c1) in fp16, and the layer-0 sqrt
            r2t = st_pool.tile([128, W], F16, tag="r2t")
            nc.scalar.activation(r2t.rearrange("p (r f) -> p r f", r=2),
                                 two_run(r2p), ACTF.Identity, bias=ca(IDX_C1))

            # ---- flow layers ---------------------------------------------
            # State: r2 biased (misses k_{l-1}); layer 0 uses r2t (true).
            r2 = None
            gp = None
            pp = None
            for l in range(L):
                r = st_pool.tile([128, W], F16, tag="r")
                if l == 0:
                    nc.scalar.activation(r.rearrange("p (r f) -> p r f", r=2),
                                         two_run(r2p), ACTF.Sqrt,
                                         bias=ca(IDX_C1))
                else:
                    nc.scalar.activation(r[:], r2[:], ACTF.Sqrt,
                                         bias=ca(IDX_K + l - 1))
                hd = st_pool.tile([128, W], F16, tag="hd")
                nc.vector.tensor_scalar(hd[:], r[:], ca(IDX_AL + l), None, A.add)
                n1 = st_pool.tile([128, W], F16, tag="n1")
                nc.vector.tensor_scalar(n1[:], r[:], ca(IDX_APB + l), None, A.add)
                g = st_pool.tile([128, W], F16, tag="g")
                nc.vector.tensor_tensor(g[:], n1[:], hd[:], A.divide)
                hp = st_pool.tile([128, W], F16, tag="hp")
                nc.vector.tensor_scalar(hp[:], hd[:], -2.0, None, A.pow)
                w_ = st_pool.tile([128, W], F16, tag="w")
                nc.vector.tensor_scalar(w_[:], hp[:], ca(IDX_AB + l), None, A.mult)

                # log-det products: pp' = (w+1)*pp, gp' = (gp*1)*g on Pool
                if l == 0:
                    u0 = st_pool.tile([128, W], F16, tag="u0")
                    nc.vector.tensor_scalar(u0[:], w_[:], 1.0, None, A.add)
                    pp = u0
                    gp = g
                else:
                    ppn = st_pool.tile([128, W], F16, tag="pp")
                    nc.gpsimd.scalar_tensor_tensor(ppn[:], w_[:], 1.0, pp[:],
                                                   A.add, A.mult)
                    gpn = st_pool.tile([128, W], F16, tag="gp")
                    nc.gpsimd.scalar_tensor_tensor(gpn[:], gp[:], 1.0, g[:],
                                                   A.mult, A.mult)
                    # u2 = gp_old*E_l + 2dd (the e-fold approximation)
                    u2 = st_pool.tile([128, W], F16, tag="u2")
                    nc.vector.tensor_tensor(u2[:], gp[:], e(l), A.mult)
                    u2b = st_pool.tile([128, W], F16, tag="u2b")
                    nc.vector.tensor_scalar(u2b[:], u2[:], ca(IDX_DD1 + l - 1),
                                            None, A.add)
                    pp = ppn
                    gp = gpn

                # t1 = (r2 + k_{l-1})*g  (layer 0: r2t already true)
                t1 = st_pool.tile([128, W], F16, tag="t1")
                if l == 0:
                    nc.vector.tensor_tensor(t1[:], g[:], r2t[:], A.mult)
                else:
                    nc.gpsimd.scalar_tensor_tensor(t1[:], r2[:],
                                                   ca(IDX_K + l - 1), g[:],
                                                   A.add, A.mult)
                t4 = st_pool.tile([128, W], F16, tag="t4")
                if l == 0:
                    nc.vector.tensor_tensor(t4[:], t1[:], e(0), A.add)
                else:
                    nc.vector.tensor_tensor(t4[:], t1[:], u2b[:], A.add)
                if l == L - 1:
                    r2n = io_pool.tile([128, W], F16, tag="r2f")
                else:
                    r2n = st_pool.tile([128, W], F16, tag="r2n")
                nc.vector.tensor_tensor(r2n[:], g[:], t4[:], A.mult)
                r2 = r2n

            # ---- outputs (host computes -0.5*(r2+k5)+15*ln gp+ln pp) ------
            nc.sync.dma_start(r2_d[sg], r2[:])
            nc.sync.dma_start(gp_d[sg], gp[:])
            nc.sync.dma_start(pp_d[sg], pp[:])

    nc.compile()
    return nc


_NC_CACHE = None


def _get_nc():
    global _NC_CACHE
    if _NC_CACHE is None:
        _NC_CACHE = _build_program()
    return _NC_CACHE


def _prepare_in_maps(z, z0, log_alpha, beta):
    blocks, blob = _host_consts(z0, log_alpha, beta)
    z = np.ascontiguousarray(z.astype(np.float32))
    in_maps = []
    for c in range(NCORES):
        shard = z[c * NC_SAMP:(c + 1) * NC_SAMP]
        pad = np.zeros((NC_PAD, DIM), np.float32)
        pad[:NC_SAMP] = shard
        # zd[g, d*8+s8, f] = z[g*(8*W) + s8*W + f, d]
        cube = pad.reshape(SGROUPS, SB, W, DIM)
        zd = np.ascontiguousarray(
            cube.transpose(0, 3, 1, 2).reshape(SGROUPS, 128, W)
        ).astype(np.float16)
        in_maps.append({"zd": zd, "wb": blocks, "cst": blob})
    return in_maps


def _finalize_core(res_map, z0, log_alpha, beta):
    """raw [SGROUPS,128=(c*8+s),W] fp16 r2/gp/pp -> [NC_SAMP, C] log-density."""
    z0 = z0.astype(np.float64)
    delta = np.concatenate([z0[:-1] - z0[1:], z0[-1:]], axis=0)
    k5 = np.sum(delta[L - 1] ** 2, axis=-1)          # [C]
    k5_col = np.repeat(k5, SB).astype(np.float32)    # [128]
    r2 = res_map["r2o"].astype(np.float32) + k5_col[None, :, None]
    gp = res_map["gpo"].astype(np.float32)
    pp = res_map["ppo"].astype(np.float32)
    out = (-0.5 * r2 + 15.0 * np.log(gp) + np.log(pp)
           - np.float32(0.5 * DIM * LOG2PI))
    # [g, c*8+s, f] -> [n, c]
    o = out.reshape(SGROUPS, C, SB, W).transpose(0, 2, 3, 1).reshape(NC_PAD, C)
    return o[:NC_SAMP]


def _numpy_fallback(z, z0, log_alpha, beta, mean, cov):
    # General mean/cov path (never hit for this problem's fixed buffers).
    z = z.astype(np.float32)
    zc = np.broadcast_to(z[None], (C,) + z.shape).astype(np.float32)
    slj = np.zeros((C, z.shape[0]), np.float32)
    alpha = np.exp(log_alpha.astype(np.float32))
    zk = zc.copy()
    for l in range(L):
        z_sub = zk - z0[l][:, None, :]
        r = np.linalg.norm(z_sub, axis=-1, keepdims=True)
        h = 1.0 / (alpha[l][:, None, None] + r)
        b = beta[l][:, None, None]
        zk = zk + b * h * z_sub
        bh = b * h
        ld = (DIM - 1) * np.log1p(bh) + np.log1p(bh - b * r * h * h)
        slj += ld[..., 0]
    Lc = np.linalg.cholesky(cov)
    diff = zk - mean[:, None, :]
    sol = np.einsum("cij,cnj->cni", np.linalg.inv(Lc), diff)
    half_logdet = np.sum(np.log(np.diagonal(Lc, axis1=-2, axis2=-1)), axis=-1)
    lpz = -0.5 * (DIM * LOG2PI + np.sum(sol * sol, axis=-1)) \
        - half_logdet[:, None]
    out = (lpz + slj).T.astype(np.float32)
    return np.where(np.isnan(out), -np.inf, out)


def kernel(z, z0, log_alpha, beta, mean, cov):
    z = np.asarray(z)
    z0 = np.asarray(z0)
    log_alpha = np.asarray(log_alpha)
    beta = np.asarray(beta)
    mean = np.asarray(mean)
    cov = np.asarray(cov)
    if (not np.all(mean == 0.0)
            or not np.array_equal(cov, np.broadcast_to(np.eye(DIM, dtype=cov.dtype),
                                                       cov.shape))):
        return _numpy_fallback(z, z0, log_alpha, beta, mean, cov)

    try:
        nc = _get_nc()
        in_maps = _prepare_in_maps(z, z0, log_alpha, beta)
        res = run_bass_kernel_spmd(nc, in_maps, list(range(NCORES)))
        outs = [_finalize_core(res.results[c], z0, log_alpha, beta)
                for c in range(NCORES)]
        out = np.concatenate(outs, axis=0).astype(np.float32)
    except Exception:
        # Device path unavailable (missing cores, wedged runtime, ...):
        # return the exact-but-slow host result instead of crashing.
        return _numpy_fallback(z, z0, log_alpha, beta, mean, cov)
    return np.where(np.isnan(out), np.float32(-np.inf), out)


# revision 7
# speedup vs baseline: 1.5799x; 1.3541x over previous
"""Trainium2 Bass kernel for nn_Density: radial-flow mixture log-density.

Computes log q(z|c) for a 6-layer batched radial normalizing flow with a
standard-normal base, for C=16 classes over N=200000 samples, data-parallel
over 8 NeuronCores.

Math: the radial update z' = z + beta*h*(z - z0) with h = 1/(alpha + r),
r = ||z - z0||, is, per (sample, class), a scalar rescaling of z_sub = z - z0:
    z_sub_{l+1} = g_l * z_sub_l + Delta_l,   g_l = 1 + beta_l*h_l,
so r^2 obeys a scalar recurrence driven by e_l = 2*z_sub.Delta_l:
    r2' = g*(g*r2 + e_l) + k_l,   k_l = ||Delta_l||^2.
The e_l recurrences are *approximated*: with B_l = prod_{i<l} g_i,
    e_l(l) ~= B_l*E_l + 2*Delta_{l-1}.Delta_l,
    E_l = 2*z_sub_0.Delta_l + 2*sum_{j<=l-2} Delta_j.Delta_l,
i.e. non-consecutive Delta.Delta cross terms are folded into the matmul
seed E_l (exact weight would be prod_{j<i<l} g_i ~= B_l); the consecutive
term keeps its exact weight 1.  Validated: max rel err ~5e-3 in fp16
(tolerance 2e-2).

Per layer: r = sqrt(r2), h = 1/(alpha+r) (fast-approx reciprocal),
    g = 1 + beta*h, u = 1 + (alpha*beta)*h^2,
    r2' = g*(g*r2 + B*E + 2dd) + k,
(log-det: slj = 15*ln(prod g) + ln(prod u)); pow/divide are not valid
DVE ISA ops, so h comes from reciprocal_approx_fast (fp32).
No reciprocal, no Ln on device: gp = prod g, pp = prod u and the final r2
are DMA'd out in fp16 and the host computes
    out = -0.5*(r2+k5) + 15*ln(gp) + ln(pp) - 8*ln(2pi).
ACT uses only Sqrt/Square/Identity -> single act table, zero table switches.

Layout: partitions hold (class, sample-block) pairs p = c*8 + s, so every
per-class constant is a per-partition scalar ([128,1] AP).  The free axis
holds W=784 samples per supergroup (4 supergroups = 25088 >= 25000 samples
per core).  Seeds come from fp16 block-sparse stationary matmuls
lhsT[(d,s8),(c,s)] = w[d,c]*delta(s8,s) over zd/zsq, written as 392-wide
bank-aligned chunk pairs in [128,1024] PSUM tiles and evacuated by single
two-run ACT copies that fold the per-class seed bias.

Engine split per layer: ACT does Sqrt (k_{l-1} rides the bias; r2 is
stored *biased*, k re-added by the t1 STT, k_5 added on host), hd = r+alpha
and h2 = Square(h); DVE does the reciprocal, the fp16 4x/2x chain ops and
gp' = gp*g; Pool/GPSIMD takes three fused ops: g = (h*beta)+ones,
t1 = (r2+k)*g and pp' = (w+1)*pp.

The program is emitted layer-major (all supergroups advance together) so
the tile scheduler can overlap the four independent dependency chains;
shared transient tags rotate with bufs=4 (one layer-row of slack).
"""

from contextlib import ExitStack

import numpy as np

import concourse.bacc as bacc
import concourse.bass as bass
import concourse.mybir as mybir
import concourse.tile as tile
from concourse.bass_utils import run_bass_kernel_spmd

F32 = mybir.dt.float32
F16 = mybir.dt.float16
A = mybir.AluOpType
ACTF = mybir.ActivationFunctionType

N, C, DIM, L = 200000, 16, 16, 6
NCORES = 8
SB = 8                      # sample blocks per class on partitions
W = 784                     # samples per partition slot per supergroup
HW_ = 392                   # matmul chunk width (bank-aligned pairs)
SGROUPS = 4
NC_SAMP = N // NCORES       # 25000
NC_PAD = SB * W * SGROUPS   # 25088

# const blob column indices ([128, NCONST] f32, value = f(class(p)))
IDX_AL = 0          # alpha_l            -> 0..5
IDX_B = 6           # beta_l             -> 6..11
IDX_AB = 12         # alpha_l * beta_l   -> 12..17
IDX_K = 18          # k_l = ||Delta_l||^2 -> 18..23
IDX_DD1 = 24        # 2*Delta_{l-1}.Delta_l, l=1..5 -> 24..28
IDX_C1 = 29         # ||z0_0||^2
IDX_EB = 30         # E_m seed bias -> 30..35
NCONST = 36

LOG2PI = float(np.log(2.0 * np.pi))


def _host_consts(z0, log_alpha, beta):
    """Build fp16 stationary blocks [8, 128, 128] and const blob [128, NCONST]."""
    z0 = z0.astype(np.float64)
    alpha = np.exp(log_alpha.astype(np.float64))
    beta = beta.astype(np.float64)
    delta = np.concatenate([z0[:-1] - z0[1:], z0[-1:]], axis=0)

    # wcols[m]: [DIM, C]; m=0 -> -2*z0_0 (r2 seed), m=1..6 -> 2*Delta_{m-1},
    # m=7 -> ones (zsq accumulation)
    wcols = np.zeros((8, DIM, C))
    wcols[0] = -2.0 * z0[0].T
    for m in range(L):
        wcols[m + 1] = 2.0 * delta[m].T
    wcols[7] = 1.0

    blocks = np.zeros((8, 128, 128), np.float16)
    eye8 = np.eye(SB)
    for j in range(8):
        blocks[j] = np.einsum("dc,st->dsct", wcols[j], eye8).reshape(128, 128)

    k = np.sum(delta ** 2, axis=-1)                        # [L, C]
    dd = np.einsum("lcd,mcd->lmc", delta, delta)           # [L, L, C]
    cst = np.zeros((NCONST, C))
    for l in range(L):
        cst[IDX_AL + l] = alpha[l]
        cst[IDX_B + l] = beta[l]
        cst[IDX_AB + l] = alpha[l] * beta[l]
        cst[IDX_K + l] = k[l]
    for l in range(1, L):
        cst[IDX_DD1 + l - 1] = 2.0 * dd[l - 1, l]
    cst[IDX_C1] = np.sum(z0[0] ** 2, axis=-1)
    for m in range(L):
        eb = -2.0 * np.einsum("cd,cd->c", z0[0], delta[m])
        if m >= 2:
            eb = eb + 2.0 * np.sum(dd[:m - 1, m], axis=0)
        cst[IDX_EB + m] = eb

    # blob[p, i] = cst[i, class(p)],  class(p) = p // 8
    blob = cst.T[np.repeat(np.arange(C), SB)].astype(np.float32).copy()
    return blocks, blob


def _build_program(reps=1):
    nc = bacc.Bacc("TRN2", target_bir_lowering=False, debug=False,
                   num_devices=NCORES)
    zd_d = nc.dram_tensor("zd", [SGROUPS, 128, W], F16, kind="ExternalInput")
    wb_d = nc.dram_tensor("wb", [8, 128, 128], F16, kind="ExternalInput")
    cst_d = nc.dram_tensor("cst", [128, NCONST], F32, kind="ExternalInput")
    r2_d = nc.dram_tensor("r2o", [SGROUPS, 128, W], F16, kind="ExternalOutput")
    gp_d = nc.dram_tensor("gpo", [SGROUPS, 128, W], F16, kind="ExternalOutput")
    pp_d = nc.dram_tensor("ppo", [SGROUPS, 128, W], F16, kind="ExternalOutput")

    with tile.TileContext(nc) as tc, ExitStack() as ctx:
        const_pool = ctx.enter_context(tc.tile_pool(name="const", bufs=1))
        wbt = const_pool.tile([128, 8 * 128], F16)
        for j in range(8):
            nc.sync.dma_start(wbt[:, j * 128:(j + 1) * 128], wb_d[j])
        cst = const_pool.tile([128, NCONST], F32)
        nc.sync.dma_start(cst[:], cst_d[:])
        ones = const_pool.tile([128, W], F16)
        nc.vector.memset(ones[:], 1.0)

        def wb(j):
            return wbt[:, j * 128:(j + 1) * 128]

        def ca(i):
            return cst[:, i:i + 1]            # [128,1] per-partition const

        io_pool = ctx.enter_context(tc.tile_pool(name="io", bufs=4))
        e_pool = ctx.enter_context(tc.tile_pool(name="e", bufs=4))
        f32_pool = ctx.enter_context(tc.tile_pool(name="f32t", bufs=4))
        st_pool = ctx.enter_context(tc.tile_pool(name="st", bufs=4))
        rot_pool = ctx.enter_context(tc.tile_pool(name="rot", bufs=8))
        psr_pool = ctx.enter_context(tc.tile_pool(name="psr", bufs=1, space="PSUM"))
        pse_pool = ctx.enter_context(tc.tile_pool(name="pse", bufs=1, space="PSUM"))

        def two_run(t):
            """[128, 1024] psum tile -> [128, 2, 392] AP (the used chunks)."""
            return t.rearrange("p (r f) -> p r f", r=2)[:, :, 0:HW_]

        for _rep in range(reps):
            e_alls = [None] * SGROUPS
            r2s = [None] * SGROUPS
            gps = [None] * SGROUPS
            pps = [None] * SGROUPS

            def e(sg, m):
                return e_alls[sg][:, m * W:(m + 1) * W]

            # ---- seeds + layer 0, one supergroup at a time ---------------
            for sg in range(SGROUPS):
                zd = io_pool.tile([128, W], F16, tag="zd")
                nc.sync.dma_start(zd[:], zd_d[sg])
                zsq = io_pool.tile([128, W], F16, tag="zsq")
                nc.scalar.activation(zsq[:], zd[:], ACTF.Square)

                r2p = psr_pool.tile([128, 1024], F32, tag="r2p")
                for h in range(2):
                    sl = slice(512 * h, 512 * h + HW_)
                    zsl = slice(HW_ * h, HW_ * (h + 1))
                    nc.tensor.matmul(r2p[:, sl], wb(0), zd[:, zsl],
                                     start=True, stop=False)
                    nc.tensor.matmul(r2p[:, sl], wb(7), zsq[:, zsl],
                                     start=False, stop=True)
                e_alls[sg] = e_pool.tile([128, L * W], F16, tag="e", name="e_all")
                for m in range(L):
                    ep = pse_pool.tile([128, 1024], F32, tag=f"ep{m % 3}")
                    for h in range(2):
                        sl = slice(512 * h, 512 * h + HW_)
                        zsl = slice(HW_ * h, HW_ * (h + 1))
                        nc.tensor.matmul(ep[:, sl], wb(m + 1), zd[:, zsl],
                                         start=True, stop=True)
                    nc.scalar.activation(
                        e(sg, m).rearrange("p (r f) -> p r f", r=2),
                        two_run(ep), ACTF.Identity, bias=ca(IDX_EB + m))

                # layer 0 (consumes r2p from PSUM, frees it early)
                r = f32_pool.tile([128, W], F32, tag="r")
                nc.scalar.activation(r.rearrange("p (r f) -> p r f", r=2),
                                     two_run(r2p), ACTF.Sqrt, bias=ca(IDX_C1))
                r2t = rot_pool.tile([128, W], F16, tag="r2")
                nc.scalar.activation(r2t.rearrange("p (r f) -> p r f", r=2),
                                     two_run(r2p), ACTF.Identity,
                                     bias=ca(IDX_C1))
                hd = f32_pool.tile([128, W], F32, tag="hd")
                nc.scalar.activation(hd[:], r[:], ACTF.Identity,
                                     bias=ca(IDX_AL))
                h_ = f32_pool.tile([128, W], F32, tag="h")
                nc.vector.reciprocal_approx_fast(h_[:], hd[:])
                g = rot_pool.tile([128, W], F16, tag="g")
                nc.gpsimd.scalar_tensor_tensor(g[:], h_[:], ca(IDX_B),
                                               ones[:], A.mult, A.add)
                h2 = st_pool.tile([128, W], F16, tag="h2")
                nc.scalar.activation(h2[:], h_[:], ACTF.Square)
                nc.vector.tensor_scalar(h2[:], h2[:], ca(IDX_AB), None, A.mult)
                u0 = rot_pool.tile([128, W], F16, tag="pp")
                nc.vector.tensor_scalar(u0[:], h2[:], 1.0, None, A.add)
                t1 = st_pool.tile([128, W], F16, tag="t1")
                nc.vector.tensor_tensor(t1[:], g[:], r2t[:], A.mult)
                nc.vector.tensor_tensor(t1[:], t1[:], e(sg, 0), A.add)
                r2n = rot_pool.tile([128, W], F16, tag="r2")
                nc.vector.tensor_tensor(r2n[:], g[:], t1[:], A.mult)
                r2s[sg] = r2n
                gps[sg] = g
                pps[sg] = u0

            # ---- layers 1..5, layer-major across supergroups -------------
            for l in range(1, L):
                for sg in range(SGROUPS):
                    r = f32_pool.tile([128, W], F32, tag="r")
                    nc.scalar.activation(r[:], r2s[sg][:], ACTF.Sqrt,
                                         bias=ca(IDX_K + l - 1))
                    hd = f32_pool.tile([128, W], F32, tag="hd")
                    nc.scalar.activation(hd[:], r[:], ACTF.Identity,
                                         bias=ca(IDX_AL + l))
                    h_ = f32_pool.tile([128, W], F32, tag="h")
                    nc.vector.reciprocal_approx_fast(h_[:], hd[:])
                    g = rot_pool.tile([128, W], F16, tag="g")
                    nc.gpsimd.scalar_tensor_tensor(g[:], h_[:],
                                                   ca(IDX_B + l), ones[:],
                                                   A.mult, A.add)
                    h2 = st_pool.tile([128, W], F16, tag="h2")
                    nc.scalar.activation(h2[:], h_[:], ACTF.Square)
                    nc.vector.tensor_scalar(h2[:], h2[:], ca(IDX_AB + l),
                                            None, A.mult)
                    # pp' = (w+1)*pp on Pool
                    ppn = rot_pool.tile([128, W], F16, tag="pp")
                    nc.gpsimd.scalar_tensor_tensor(ppn[:], h2[:], 1.0,
                                                   pps[sg][:], A.add, A.mult)
                    # u2 = gp_old*E_l + 2dd
                    u2 = st_pool.tile([128, W], F16, tag="u2")
                    nc.vector.tensor_tensor(u2[:], gps[sg][:], e(sg, l),
                                            A.mult)
                    nc.vector.tensor_scalar(u2[:], u2[:],
                                            ca(IDX_DD1 + l - 1), None, A.add)
                    # gp' = gp*g on DVE
                    gpn = rot_pool.tile([128, W], F16, tag="gp")
                    nc.vector.tensor_tensor(gpn[:], gps[sg][:], g[:], A.mult)
                    # t1 = (r2 + k_{l-1})*g on Pool (re-adds the sqrt bias)
                    t1 = st_pool.tile([128, W], F16, tag="t1")
                    nc.gpsimd.scalar_tensor_tensor(t1[:], r2s[sg][:],
                                                   ca(IDX_K + l - 1), g[:],
                                                   A.add, A.mult)
                    nc.vector.tensor_tensor(u2[:], t1[:], u2[:], A.add)
                    r2n = rot_pool.tile([128, W], F16, tag="r2")
                    nc.vector.tensor_tensor(r2n[:], g[:], u2[:], A.mult)
                    r2s[sg] = r2n
                    gps[sg] = gpn
                    pps[sg] = ppn

            # ---- outputs (host computes -0.5*(r2+k5)+15*ln gp+ln pp) -----
            for sg in range(SGROUPS):
                nc.sync.dma_start(r2_d[sg], r2s[sg][:])
                nc.sync.dma_start(gp_d[sg], gps[sg][:])
                nc.sync.dma_start(pp_d[sg], pps[sg][:])

    nc.compile()
    return nc


_NC_CACHE = None


def _get_nc():
    global _NC_CACHE
    if _NC_CACHE is None:
        _NC_CACHE = _build_program()
    return _NC_CACHE


def _prepare_in_maps(z, z0, log_alpha, beta):
    blocks, blob = _host_consts(z0, log_alpha, beta)
    z = np.ascontiguousarray(z.astype(np.float32))
    in_maps = []
    for c in range(NCORES):
        shard = z[c * NC_SAMP:(c + 1) * NC_SAMP]
        pad = np.zeros((NC_PAD, DIM), np.float32)
        pad[:NC_SAMP] = shard
        # zd[g, d*8+s8, f] = z[g*(8*W) + s8*W + f, d]
        cube = pad.reshape(SGROUPS, SB, W, DIM)
        zd = np.ascontiguousarray(
            cube.transpose(0, 3, 1, 2).reshape(SGROUPS, 128, W)
        ).astype(np.float16)
        in_maps.append({"zd": zd, "wb": blocks, "cst": blob})
    return in_maps


def _finalize_core(res_map, z0, log_alpha, beta):
    """raw [SGROUPS,128=(c*8+s),W] fp16 r2/gp/pp -> [NC_SAMP, C] log-density."""
    z0 = z0.astype(np.float64)
    delta = np.concatenate([z0[:-1] - z0[1:], z0[-1:]], axis=0)
    k5 = np.sum(delta[L - 1] ** 2, axis=-1)          # [C]
    k5_col = np.repeat(k5, SB).astype(np.float32)    # [128]
    r2 = res_map["r2o"].astype(np.float32) + k5_col[None, :, None]
    gp = res_map["gpo"].astype(np.float32)
    pp = res_map["ppo"].astype(np.float32)
    out = (-0.5 * r2 + 15.0 * np.log(gp) + np.log(pp)
           - np.float32(0.5 * DIM * LOG2PI))
    # [g, c*8+s, f] -> [n, c]
    o = out.reshape(SGROUPS, C, SB, W).transpose(0, 2, 3, 1).reshape(NC_PAD, C)
    return o[:NC_SAMP]


def _numpy_fallback(z, z0, log_alpha, beta, mean, cov):
    # General mean/cov path (never hit for this problem's fixed buffers).
    z = z.astype(np.float32)
    zc = np.broadcast_to(z[None], (C,) + z.shape).astype(np.float32)
    slj = np.zeros((C, z.shape[0]), np.float32)
    alpha = np.exp(log_alpha.astype(np.float32))
    zk = zc.copy()
    for l in range(L):
        z_sub = zk - z0[l][:, None, :]
        r = np.linalg.norm(z_sub, axis=-1, keepdims=True)
        h = 1.0 / (alpha[l][:, None, None] + r)
        b = beta[l][:, None, None]
        zk = zk + b * h * z_sub
        bh = b * h
        ld = (DIM - 1) * np.log1p(bh) + np.log1p(bh - b * r * h * h)
        slj += ld[..., 0]
    Lc = np.linalg.cholesky(cov)
    diff = zk - mean[:, None, :]
    sol = np.einsum("cij,cnj->cni", np.linalg.inv(Lc), diff)
    half_logdet = np.sum(np.log(np.diagonal(Lc, axis1=-2, axis2=-1)), axis=-1)
    lpz = -0.5 * (DIM * LOG2PI + np.sum(sol * sol, axis=-1)) \
        - half_logdet[:, None]
    out = (lpz + slj).T.astype(np.float32)
    return np.where(np.isnan(out), -np.inf, out)


def kernel(z, z0, log_alpha, beta, mean, cov):
    z = np.asarray(z)
    z0 = np.asarray(z0)
    log_alpha = np.asarray(log_alpha)
    beta = np.asarray(beta)
    mean = np.asarray(mean)
    cov = np.asarray(cov)
    if (not np.all(mean == 0.0)
            or not np.array_equal(cov, np.broadcast_to(np.eye(DIM, dtype=cov.dtype),
                                                       cov.shape))):
        return _numpy_fallback(z, z0, log_alpha, beta, mean, cov)

    try:
        nc = _get_nc()
        in_maps = _prepare_in_maps(z, z0, log_alpha, beta)
        res = run_bass_kernel_spmd(nc, in_maps, list(range(NCORES)))
        outs = [_finalize_core(res.results[c], z0, log_alpha, beta)
                for c in range(NCORES)]
        out = np.concatenate(outs, axis=0).astype(np.float32)
    except Exception:
        # Device path unavailable (missing cores, wedged runtime, ...):
        # return the exact-but-slow host result instead of crashing.
        return _numpy_fallback(z, z0, log_alpha, beta, mean, cov)
    return np.where(np.isnan(out), np.float32(-np.inf), out)
